# revision 38
# baseline (speedup 1.0000x reference)
"""CapsLayer2D dynamic-routing kernel for 8x TRN2 NeuronCores.

Problem (hardcoded shapes):
  inputs: [B=16, R=8, C=8, I=128, DIN=16] fp32
  W:      [K=32, I=128, DIN=16, DOUT=16] fp32
  out:    [B, R, C, K, DOUT] fp32

Math (3-round dynamic routing, closed form, verified vs reference):
  U[p,k]    = res[p,k,:,:]  (I x O per position p=(b,r,c) and k)
  s0        = mean_i U_i ; v0 = squash(s0)
  t_a = U v0 ; m_a = U^T t_a ; s1 = s0 + m_a ; v1 = squash(s1)
  t_b = U (v0+v1) ; m_b = U^T t_b ; s2 = s0 + m_b ; out = squash(s2)

Sharding: batch across 8 cores (128 positions/core), W replicated.

Performance design:
  - All W/X layout work (pad d 16->32, transpose to matmul operand
    layout, fp32->fp16 cast) is host-side numpy: zero device prep.
  - 4 k-groups of 8 caps. Production per group: res via 128 per-i
    matmuls (tile_position quadrants), one full PSUM bank per matmul
    (concurrent start/stop groups must not share a bank), strided
    cross-bank evictions on Act; 3 PSUM buffers so the eviction stream
    never ping-pong-stalls against the PE. s0 via 32 accumulating
    matmuls, emitted after res so the DVE unblocks earliest.
  - Routing on the DVE with TENSOR_TENSOR only (2x mode: fp16,
    unit-stride innermost; TRN2 has no 4x for two-stream ops, and
    tensor_reduce has no perf modes at all). Contractions are log2
    trees over sliced views, computed in place inside one scratch
    tile. The U^T t contraction reads t through a duplicated-pair
    tile t2[p,k,i,2] built by a single butterfly add (reversed-stride
    operand), keeping every operand's innermost AP packed.
  - Two groups are software-pipelined; squash/mid/out small ops are
    batched pair-wide ([P,256]) to halve instruction count and Act
    round trips. v0's squash stays per-group (startup path).
"""

import sys

import numpy as np

sys.path.insert(0, "/opt/trn_rl_repo")

P, I, D, K, O = 128, 128, 16, 32, 16
D2 = 32  # padded d
ID = I * D  # 2048
KO = K * O  # 512
KC = 8  # k-group size
NG = K // KC  # 4 groups
GW = KC * O  # 128 group output width
GKO = 32 * KC * O  # per-group W cols: 32 chunks x (k8,o16) = 4096
N_CORES = 8
EPS = 1e-7

_PROGRAM = None


def _build_program():
    from contextlib import ExitStack

    import concourse.tile as tile
    from concourse import bacc, mybir

    F32 = mybir.dt.float32
    F16 = mybir.dt.float16
    ADD = mybir.AluOpType.add
    MULT = mybir.AluOpType.mult
    X = mybir.AxisListType.X
    SQRT = mybir.ActivationFunctionType.Sqrt

    nc = bacc.Bacc("TRN2", target_bir_lowering=False, debug=False)

    xt_d = nc.dram_tensor("xt", [128, 32 * 128], F16, kind="ExternalInput").ap()
    wr_d = nc.dram_tensor("wr", [128, NG * GKO], F16, kind="ExternalInput").ap()
    out_d = nc.dram_tensor("out", [P, KO], F32, kind="ExternalOutput").ap()

    with ExitStack() as ctx:
        tc = ctx.enter_context(tile.TileContext(nc))

        pp_s = ctx.enter_context(tc.tile_pool(name="pp_s", bufs=2, space="PSUM"))
        pp_r = ctx.enter_context(tc.tile_pool(name="pp_r", bufs=3, space="PSUM"))

        xp = ctx.enter_context(tc.tile_pool(name="xt", bufs=1))
        wp = ctx.enter_context(tc.tile_pool(name="wr", bufs=1))
        rp = ctx.enter_context(tc.tile_pool(name="res", bufs=3))
        sp = ctx.enter_context(tc.tile_pool(name="scratch", bufs=1))
        sm = ctx.enter_context(tc.tile_pool(name="small", bufs=1))

        Xt = xp.tile([128, 32 * 128], F16)

        def dma_xt(q):
            nc.sync.dma_start(
                Xt[:, q * 1024:(q + 1) * 1024],
                xt_d[:, q * 1024:(q + 1) * 1024],
            )

        dma_xt(0)

        eps_t = sm.tile([P, 1], F32, tag="eps")
        nc.vector.memset(eps_t[:], EPS)

        state = {g: {} for g in range(NG)}

        KP = 2 * KC  # caps per pair of groups
        PW = 2 * GW

        def squash_n(key, s_ap, v_ap, kn):
            """v = squash(s); fp32 [P, (kn,o16)]; sqrt on Act."""
            ssq = sm.tile([P, kn * O], F32, tag=f"ssq_{key}")
            nc.vector.tensor_mul(ssq[:], s_ap, s_ap)
            sq = sm.tile([P, kn], F32, tag=f"sq_{key}")
            nc.vector.tensor_reduce(
                sq[:], ssq[:].rearrange("p (k o) -> p k o", k=kn), X, ADD
            )
            a = sm.tile([P, kn], F32, tag=f"sqa_{key}")
            nc.scalar.activation(a[:], sq[:], SQRT, bias=eps_t[:])
            b = sm.tile([P, kn], F32, tag=f"sqb_{key}")
            nc.vector.scalar_tensor_tensor(b[:], sq[:], 1.0, a[:], ADD, MULT)
            r = sm.tile([P, kn], F32, tag=f"sqr_{key}")
            nc.vector.reciprocal(r[:], b[:])
            f = sm.tile([P, kn], F32, tag=f"sqf_{key}")
            nc.vector.tensor_mul(f[:], sq[:], r[:])
            nc.vector.tensor_mul(
                v_ap.rearrange("p (k o) -> p k o", k=kn),
                s_ap.rearrange("p (k o) -> p k o", k=kn),
                f[:].unsqueeze(2).broadcast_to([P, kn, O]),
            )

        def produce(g):
            st = state[g]
            pr = g % 2
            W_g = wp.tile([128, GKO], F16, tag="wg")
            for q in range(4):
                nc.sync.dma_start(
                    W_g[:, q * 1024:(q + 1) * 1024],
                    wr_d[:, g * GKO + q * 1024:g * GKO + (q + 1) * 1024],
                )
                if g == 0 and q < 3:
                    # interleave remaining xt chunks behind wr chunks so
                    # the first res matmul's operands land earliest
                    dma_xt(q + 1)

            res = rp.tile([P, KC * I * O], F16, tag="res")
            resv = res[:].rearrange("p (k i o) -> p k i o", k=KC, i=I, o=O)
            for c in range(32):
                for m in range(2):
                    prb = pp_r.tile([P, 1024], F32, tag="prb")
                    for j in (2 * m, 2 * m + 1):
                        r0 = j * 32
                        nc.tensor.matmul(
                            prb[:, (j % 2) * 512:(j % 2) * 512 + GW],
                            Xt[r0:r0 + 32, c * 128:(c + 1) * 128],
                            W_g[r0:r0 + 32, c * 128:(c + 1) * 128],
                            start=True,
                            stop=True,
                            tile_position=(r0, 0),
                        )
                    src = prb[:].rearrange("p (i x) -> p i x", i=2)[
                        :, :, 0:GW
                    ].rearrange("p i (k o) -> p i k o", k=KC)
                    dst = resv[
                        :, :, 4 * c + 2 * m:4 * c + 2 * m + 2, :
                    ].transpose([0, 2, 1, 3])
                    # groups 0/1: DVE is idle before its first work, so
                    # splitting evictions shortens the startup path
                    if g == 0 and m == 1:
                        nc.vector.tensor_copy(dst, src)
                    else:
                        nc.scalar.copy(dst, src)

            # s0 after res: the PE runs res matmuls first so the DVE's
            # first evictions/routing unblock as early as possible
            ps0 = pp_s.tile([P, 512], F32, tag="ps0")
            for c in range(32):
                nc.tensor.matmul(
                    ps0[:, 0:GW],
                    Xt[:, c * 128:(c + 1) * 128],
                    W_g[:, c * 128:(c + 1) * 128],
                    start=(c == 0),
                    stop=(c == 31),
                )
            pq = (g // 2) % 2
            half = g % 2
            # one pair tile, created by the half-0 producer and shared with
            # the half-1 producer (a fresh .tile() per call would version
            # the tile and leave the other half untracked/unordered)
            if half == 0:
                s0p = sm.tile([P, PW], F32, tag=f"s0p{pq}")
            else:
                s0p = state[g - 1]["s0p"]
            s0h = s0p[:, half * GW:(half + 1) * GW]
            nc.scalar.mul(s0h, ps0[:, 0:GW], 1.0 / I)
            v0 = sm.tile([P, GW], F32, tag=f"v0_{half}")
            squash_n(f"v0_{half}", s0h, v0[:], KC)
            v0h = sm.tile([P, GW], F16, tag=f"v0h_{half}")
            nc.vector.tensor_copy(v0h[:], v0[:])

            st["res"] = res
            st["s0p"] = s0p
            st["v0"] = v0
            st["v0h"] = v0h[:]

        def S_mid_pair(A, B):
            """Batched: s1 = s0 + m_a; v1 = squash(s1); vsh = fp16(v0+v1)."""
            pq = (A // 2) % 2
            s0p = state[A]["s0p"]
            mp = state[A]["mp"]
            s1p = sm.tile([P, PW], F32, tag=f"s1p{pq}")
            nc.vector.tensor_add(s1p[:], s0p[:], mp[:])
            v1p = sm.tile([P, PW], F32, tag=f"v1p{pq}")
            squash_n(f"v1_{pq}", s1p[:], v1p[:], KP)
            vsp = sm.tile([P, PW], F32, tag=f"vsp{pq}")
            for g, half in ((A, 0), (B, 1)):
                nc.vector.tensor_add(
                    vsp[:, half * GW:(half + 1) * GW],
                    state[g]["v0"][:],
                    v1p[:, half * GW:(half + 1) * GW],
                )
            vshp = sm.tile([P, PW], F16, tag=f"vshp{pq}")
            nc.vector.tensor_copy(vshp[:], vsp[:])
            for g, half in ((A, 0), (B, 1)):
                state[g]["vsh"] = vshp[:, half * GW:(half + 1) * GW]

        def S_out_pair(A, B):
            """Batched: s2 = s0 + m_b; out = squash(s2); one pair DMA."""
            pq = (A // 2) % 2
            s2p = sm.tile([P, PW], F32, tag=f"s2p{pq}")
            nc.vector.tensor_add(s2p[:], state[A]["s0p"][:], state[A]["mp"][:])
            outtp = sm.tile([P, PW], F32, tag=f"outtp{pq}")
            squash_n(f"out_{pq}", s2p[:], outtp[:], KP)
            nc.sync.dma_start(out_d[:, A * GW:(A + 2) * GW], outtp[:])

        def S_out_one(g):
            """Per-group out (tail of the last pair: DMA as early as
            possible instead of waiting for the pair partner)."""
            half = g % 2
            s0h = state[g]["s0p"][:, half * GW:(half + 1) * GW]
            mh = state[g]["mp"][:, half * GW:(half + 1) * GW]
            s2 = sm.tile([P, GW], F32, tag=f"s2o{half}")
            nc.vector.tensor_add(s2[:], s0h, mh)
            outt = sm.tile([P, GW], F32, tag=f"outto{half}")
            squash_n(f"outo_{half}", s2[:], outt[:], KC)
            nc.sync.dma_start(out_d[:, g * GW:(g + 1) * GW], outt[:])

        # k-split between engines: Pool owns capsule k=0 of every pass
        # end-to-end (measured ~17.5us/pass, fits inside the DVE's ~31us
        # 7/8-slice); the slices only merge at the m write.
        KPOOL = 1

        def S_uv(g, vkey, split=False):
            """tmp = res * v (bcast over i); in-place o-tree 16 -> 4."""
            st = state[g]
            tmp = sp.tile([P, KC * I * O], F16, tag=f"tmp{g % 2}")
            st["tmp"] = tmp
            t4 = tmp[:].rearrange("p (k i o) -> p k i o", k=KC, i=I)
            rv4 = st["res"][:].rearrange("p (k i o) -> p k i o", k=KC, i=I)
            vb4 = (
                st[vkey]
                .rearrange("p (k o) -> p k o", k=KC)
                .unsqueeze(2)
                .broadcast_to([P, KC, I, O])
            )
            kp = KPOOL
            nc.gpsimd.tensor_mul(t4[:, 0:kp], rv4[:, 0:kp], vb4[:, 0:kp])
            nc.gpsimd.tensor_add(
                t4[:, 0:kp, :, 0:8], t4[:, 0:kp, :, 0:8], t4[:, 0:kp, :, 8:16]
            )
            nc.gpsimd.tensor_add(
                t4[:, 0:kp, :, 0:4], t4[:, 0:kp, :, 0:4], t4[:, 0:kp, :, 4:8]
            )
            if split:
                # startup only: the first i-half is ready after half the
                # evictions, so the mul can begin ~5us earlier
                h = I // 2
                nc.vector.tensor_mul(
                    t4[:, kp:, 0:h], rv4[:, kp:, 0:h], vb4[:, kp:, 0:h]
                )
                nc.vector.tensor_mul(
                    t4[:, kp:, h:I], rv4[:, kp:, h:I], vb4[:, kp:, h:I]
                )
            else:
                nc.vector.tensor_mul(t4[:, kp:], rv4[:, kp:], vb4[:, kp:])
            nc.vector.tensor_add(
                t4[:, kp:, :, 0:8], t4[:, kp:, :, 0:8], t4[:, kp:, :, 8:16]
            )
            nc.vector.tensor_add(
                t4[:, kp:, :, 0:4], t4[:, kp:, :, 0:4], t4[:, kp:, :, 4:8]
            )

        def S_otail(g):
            """In-place o-tree 4 -> 2 (both engines, own k-slices)."""
            t4 = state[g]["tmp"][:].rearrange("p (k i o) -> p k i o", k=KC, i=I)
            kp = KPOOL
            nc.gpsimd.tensor_add(
                t4[:, 0:kp, :, 0:2], t4[:, 0:kp, :, 0:2], t4[:, 0:kp, :, 2:4]
            )
            nc.vector.tensor_add(
                t4[:, kp:, :, 0:2], t4[:, kp:, :, 0:2], t4[:, kp:, :, 2:4]
            )

        def S_btf(g):
            """Butterfly -> both t2 slots get the o-pair sum."""
            st = state[g]
            t2 = sp.tile([P, KC * I * 2], F16, tag=f"t2{g % 2}")
            st["t2"] = t2
            r2v = (
                state[g]["tmp"][:]
                .rearrange("p (k i o) -> p k i o", k=KC, i=I)[:, :, :, 0:2]
            )
            t2v = t2[:].rearrange("p (k i two) -> p k i two", k=KC, i=I)
            kp = KPOOL
            nc.gpsimd.tensor_add(
                t2v[:, 0:kp], r2v[:, 0:kp], r2v[:, 0:kp, :, ::-1]
            )
            nc.vector.tensor_add(
                t2v[:, kp:], r2v[:, kp:], r2v[:, kp:, :, ::-1]
            )

        def S_ut(g):
            """tmp = res * t2-pairs; in-place i-tree 128 -> 32."""
            st = state[g]
            tmp = sp.tile([P, KC * I * O], F16, tag=f"tmp{g % 2}")
            st["tmp"] = tmp
            t5 = tmp[:].rearrange(
                "p (k i o2 two) -> p k i o2 two", k=KC, i=I, o2=O // 2
            )
            rv5 = st["res"][:].rearrange(
                "p (k i o2 two) -> p k i o2 two", k=KC, i=I, o2=O // 2
            )
            tb5 = (
                st["t2"][:]
                .rearrange("p (k i two) -> p k i two", k=KC, i=I)
                .unsqueeze(3)
                .broadcast_to([P, KC, I, O // 2, 2])
            )
            t4 = tmp[:].rearrange("p (k i o) -> p k i o", k=KC, i=I)
            kp = KPOOL
            nc.gpsimd.tensor_mul(t5[:, 0:kp], rv5[:, 0:kp], tb5[:, 0:kp])
            nc.vector.tensor_mul(t5[:, kp:], rv5[:, kp:], tb5[:, kp:])
            for eng, ks in ((nc.gpsimd, slice(0, kp)), (nc.vector, slice(kp, KC))):
                eng.tensor_add(
                    t4[:, ks, 0:64, :], t4[:, ks, 0:64, :], t4[:, ks, 64:128, :]
                )
                eng.tensor_add(
                    t4[:, ks, 0:32, :], t4[:, ks, 0:32, :], t4[:, ks, 32:64, :]
                )
                eng.tensor_add(
                    t4[:, ks, 0:16, :], t4[:, ks, 0:16, :], t4[:, ks, 16:32, :]
                )

        def S_itail(g):
            """In-place i-tree 16 -> 2, then m = row0 + row1."""
            st = state[g]
            t4 = st["tmp"][:].rearrange("p (k i o) -> p k i o", k=KC, i=I)
            kp = KPOOL
            pq = (g // 2) % 2
            half = g % 2
            if half == 0:
                mp = sm.tile([P, PW], F16, tag=f"mp{pq}")
                for gg in (g, g + 1):
                    state[gg]["mp"] = mp
            else:
                mp = state[g]["mp"]
            mv = mp[:, half * GW:(half + 1) * GW].rearrange(
                "p (k o) -> p k o", k=KC
            )
            for eng, ks in ((nc.gpsimd, slice(0, kp)), (nc.vector, slice(kp, KC))):
                n = 16
                while n > 2:
                    h = n // 2
                    eng.tensor_add(
                        t4[:, ks, 0:h, :], t4[:, ks, 0:h, :], t4[:, ks, h:n, :]
                    )
                    n = h
                eng.tensor_add(mv[:, ks], t4[:, ks, 0, :], t4[:, ks, 1, :])

        with nc.allow_low_precision(reason="fp16 routing intermediates"):
            produce(0)
            produce(1)
            for A, B in ((0, 1), (2, 3)):
                S_uv(A, "v0h", split=(A == 0)); S_otail(A)
                S_uv(B, "v0h"); S_otail(B)
                S_btf(A); S_ut(A); S_itail(A)
                S_btf(B); S_ut(B); S_itail(B)
                S_mid_pair(A, B)
                S_uv(A, "vsh"); S_otail(A)
                S_uv(B, "vsh"); S_otail(B)
                S_btf(A); S_ut(A); S_itail(A)
                if A == 0:
                    produce(2)
                S_btf(B); S_ut(B); S_itail(B)
                if A == 0:
                    produce(3)
                    S_out_pair(A, B)
                else:
                    S_out_one(A)
                    S_out_one(B)

    nc.compile()
    return nc


def _get_program():
    global _PROGRAM
    if _PROGRAM is None:
        _PROGRAM = _build_program()
    return _PROGRAM


def _make_in_maps(inputs):
    x = np.ascontiguousarray(np.asarray(inputs["inputs"], dtype=np.float32))
    W = np.ascontiguousarray(np.asarray(inputs["W"], dtype=np.float32))
    assert x.shape == (16, 8, 8, 128, 16) and W.shape == (32, 128, 16, 16)

    # xt rows: (i%4)*32 + d, cols: (i//4)*128 + p  (d padded 16->32)
    xs = x.reshape(N_CORES, P, I, D)  # [core, p, i, d]
    xt = np.zeros((N_CORES, 4, D2, 32, P), np.float32)
    # [core, i4, d, c, p] <- [core, c, i4, d, p]
    xt[:, :, 0:D] = xs.reshape(N_CORES, P, 32, 4, D).transpose(0, 3, 4, 2, 1)
    xt = xt.reshape(N_CORES, 128, 32 * 128).astype(np.float16)

    # wr rows: (i%4)*32 + d, cols: g*4096 + (i//4)*128 + (k%8)*16 + o
    wv = W.reshape(NG, KC, 32, 4, D, O)  # [g, k8, c, i4, d, o]
    wr = np.zeros((4, D2, NG, 32, KC, O), np.float32)  # [i4, d, g, c, k8, o]
    wr[:, 0:D] = wv.transpose(3, 4, 0, 2, 1, 5)
    wr = np.ascontiguousarray(
        wr.reshape(128, NG * GKO).astype(np.float16)
    )

    return [
        {"xt": np.ascontiguousarray(xt[c]), "wr": wr} for c in range(N_CORES)
    ]


def kernel(**inputs):
    from concourse.bass_utils import run_bass_kernel_spmd

    nc = _get_program()
    in_maps = _make_in_maps(inputs)
    r = run_bass_kernel_spmd(nc, in_maps, list(range(N_CORES)))
    outs = [r.results[c]["out"].reshape(2, 8, 8, K, O) for c in range(N_CORES)]
    return np.concatenate(outs, axis=0).astype(np.float32)
